# revision 13
# baseline (speedup 1.0000x reference)
# Trainium2 Bass kernel for AtomTypeGNN message passing.
#
#   adj_exp[m,k] = sum_n dist_adj[m,n] * dist_exp[m,n,k]          (streams 1 GiB)
#   feat[m,o]    = sum_{f,h} adj_exp[m,f] * w[f,h,o] * emb[m,h]
#   out          = softplus(feat) + b
#
# Output row m depends only on row m of the inputs -> pure data parallel over
# atoms, 8 NeuronCores, 256 atoms each, no collectives.
#
# Device strategy (per core), v2:
#   Step 1: atoms stream in groups of 8; one 2 MiB DMA per group on the sync
#     (SP HWDGE) queue, which carries NOTHING else, so no cross-phase
#     dependency ever stalls the exp stream (v1 lost ~15us to the aexp load
#     parked between blocks in the SP FIFO).  All 8 atoms of a group
#     accumulate into ONE PSUM bank at different free offsets: start=True on
#     the group's first matmul marks the whole 2 KiB zero region pending-zero,
#     so each atom's first chunk writes and later chunks accumulate
#     (has_written semantics).  One ScalarE evac [1,512] per group replaces
#     v1's four [1,64] copies (ScalarE busy ~115us -> ~22us).
#   Step 2: per 128-atom block, ONE scratch store + ONE gather load (both on
#     the gpsimd SWDGE queue) transpose adj_exp to [128 atoms, 64].  The
#     f-contraction feat = sum_f aexp[:,f] * G_f runs as FOUR interleaved
#     DVE scalar_tensor_tensor chains (dep distance 4 hides DVE latency);
#     G_f = emb @ w[f] is computed on the PE during the stream.  Softplus
#     splits between ScalarE (abs/exp/ln in one act table, pre-warmed at
#     kernel start so no table load lands in the tail) and DVE (min/relu/
#     adds).  Output DMAs ride the scalar (ACT HWDGE) queue: gpsimd's
#     end-of-kernel DRAIN cost ~7us in v1.
#
# Inputs are pre-swizzled/cast on the host (bf16 compute, f32 accumulate:
# ~3e-3 relative error, memory roofline halves to ~190us/core).

import numpy as np
import ml_dtypes

N = 2048
K = 64
H = 128
OUT = 128
N_CORES = 8
M = N // N_CORES  # 256 atoms per core
GA = 8            # atoms per group / per PSUM bank
NG = M // GA      # 32 groups per core
NBLK = M // 128   # 2 step-2 blocks per core

_BF = ml_dtypes.bfloat16

_CACHE = {}


def _ensure_path():
    import sys

    for p in ("/opt/trn_rl_repo",):
        if p not in sys.path:
            sys.path.insert(0, p)


def _build():
    _ensure_path()
    import concourse.bass as bass  # noqa: F401
    import concourse.tile as tile
    from concourse import bacc, mybir

    f32 = mybir.dt.float32
    bf16 = mybir.dt.bfloat16
    fp16 = mybir.dt.float16

    nc = bacc.Bacc(
        "TRN2",
        target_bir_lowering=False,
        debug=False,
        num_devices=N_CORES,
    )

    # [t, p, aq]: atom group t = atoms 8t..8t+7, partition p, aq = 1024*a + q,
    # q = 64*c + k, n = 16p + c.  Per partition 16 KiB contiguous in DRAM.
    exp_d = nc.declare_dram_parameter("exp", [NG, 128, 8 * 1024], bf16, isOutput=False)
    # adjA[j, 16m + c] = dist_adj[m, 16j + c]
    adjA_d = nc.declare_dram_parameter("adjA", [128, 16 * M], bf16, isOutput=False)
    # embT[h, m]
    embT_d = nc.declare_dram_parameter("embT", [H, M], bf16, isOutput=False)
    # w2[h, 128f + o] = bilinear_w[f, h, o]
    w_d = nc.declare_dram_parameter("w", [H, K * OUT], bf16, isOutput=False)
    # bias broadcast to all partitions
    bias_d = nc.declare_dram_parameter("bias", [128, OUT], f32, isOutput=False)
    out_d = nc.declare_dram_parameter("out", [M, OUT], f32, isOutput=True)

    # adj_exp bounce buffer, one row per block: [g*512 + a*64 + k]
    scratch_d = nc.dram_tensor("scratch", [NBLK, 16 * 512], fp16)

    with tile.TileContext(nc) as tc:
        with (
            tc.tile_pool(name="const", bufs=1) as constp,
            tc.tile_pool(name="exp", bufs=6) as expp,
            tc.tile_pool(name="ps1", bufs=5, space="PSUM") as ps1p,
            tc.tile_pool(name="stage", bufs=1) as stagep,
            tc.tile_pool(name="aexp", bufs=2) as aexpp,
            tc.tile_pool(name="ps2", bufs=3, space="PSUM") as ps2p,
            tc.tile_pool(name="gsb", bufs=2) as gsbp,
            tc.tile_pool(name="acc", bufs=10) as accp,
            tc.tile_pool(name="outp", bufs=6) as outp,
        ):
            # consts at the HEAD of the sync queue: they must land at full
            # rate before the stream floods HBM (on the scalar queue they
            # trickled at ~70 GB/s against the saturated stream and the PE
            # sat idle 20us waiting for adjA).
            biassb = constp.tile([128, OUT], f32, tag="bias")
            nc.sync.dma_start(biassb[:], bias_d[:, :])
            adjA = constp.tile([128, 16 * M], bf16, tag="adjA")
            nc.sync.dma_start(adjA[:], adjA_d[:, :])
            wsb = constp.tile([128, K * OUT], bf16, tag="wsb")
            nc.sync.dma_start(wsb[:], w_d[:, :])
            embT = constp.tile([128, M], bf16, tag="embT")
            nc.sync.dma_start(embT[:], embT_d[:, :])

            # Warm the natural_log_exp act table (abs/exp/ln/relu/copy share
            # it) before the first evac copy, so no ACT_TABLE_LOAD lands in
            # the tail's critical path.
            warm = constp.tile([1, 2], f32, tag="warm")
            nc.scalar.activation(
                warm[0:1, :], biassb[0:1, 0:2], mybir.ActivationFunctionType.Abs
            )

            # G pre-phase: ALL G_f = emb @ w[f] for both blocks, before the
            # stream.  G only needs embT/wsb, and the PE is otherwise idle
            # while the exp prefetch fills its buffers.  Interleaving G with
            # the stream (v2) head-of-line-blocked the in-order PE queue on
            # ps2 evacuation, and the per-block chain head-of-line-blocked
            # the in-order DVE queue ahead of the next block's casts — a
            # self-amplifying stall that cost ~30us.
            gsbs = []
            for blk in range(NBLK):
                gsb = gsbp.tile([128, K * OUT], fp16, tag="gsb")
                gsbs.append(gsb)
                for f in range(K):
                    g2 = ps2p.tile([128, OUT], f32, tag="ps2")
                    nc.tensor.matmul(
                        g2[:, :],
                        embT[:, 128 * blk : 128 * (blk + 1)],
                        wsb[:, OUT * f : OUT * (f + 1)],
                        start=True,
                        stop=True,
                    )
                    nc.vector.tensor_copy(gsb[:, OUT * f : OUT * (f + 1)], g2[:, :])

            for blk in range(NBLK):
                gsb = gsbs[blk]
                # 16 group stages, each [1, 512] = 8 atoms x 64 k
                stage = stagep.tile([1, 16 * 512], fp16, tag="stage")

                for g in range(16):
                    t = blk * 16 + g
                    et = expp.tile([128, 8 * 1024], bf16, tag="exp")
                    nc.sync.dma_start(et[:], exp_d[t])
                    # 8 atoms share one PSUM bank; single start clears the
                    # whole 2 KiB zero region, per-address has_written turns
                    # each atom's first chunk into a write.
                    ps = ps1p.tile([128, 512], f32, tag="ps1")
                    for a in range(GA):
                        m = t * GA + a
                        for c in range(16):
                            nc.tensor.matmul(
                                ps[0:1, 64 * a : 64 * (a + 1)],
                                adjA[:, 16 * m + c : 16 * m + c + 1],
                                et[:, 1024 * a + 64 * c : 1024 * a + 64 * (c + 1)],
                                start=(a == 0 and c == 0),
                                stop=(a == GA - 1 and c == 15),
                            )
                    nc.scalar.copy(stage[0:1, 512 * g : 512 * (g + 1)], ps[0:1, :])

                # ---- step 2 for this block of 128 atoms ----
                # transpose bounce through DRAM on the gpsimd queue (the sync
                # queue must stay pure exp stream)
                nc.gpsimd.dma_start(scratch_d[blk : blk + 1, :], stage[0:1, :])
                # f32: DVE tensor_scalar ops require a float32 scalar operand;
                # the SWDGE load casts fp16 -> f32 in flight.
                aexp = aexpp.tile([128, K], f32, tag="aexp")
                nc.gpsimd.dma_start(
                    aexp[:],
                    scratch_d[blk : blk + 1, :].rearrange("b (p k) -> (b p) k", k=K),
                )
                # four interleaved DVE scale-accumulate chains over f
                accs = [None] * 4
                for r in range(16):
                    for ci in range(4):
                        f = 4 * r + ci
                        nacc = accp.tile([128, OUT], fp16, tag=f"acc{ci}")
                        if r == 0:
                            nc.vector.tensor_scalar_mul(
                                nacc[:], gsb[:, OUT * f : OUT * (f + 1)],
                                aexp[:, f : f + 1],
                            )
                        else:
                            nc.vector.scalar_tensor_tensor(
                                nacc[:],
                                gsb[:, OUT * f : OUT * (f + 1)],
                                aexp[:, f : f + 1],
                                accs[ci][:],
                                mybir.AluOpType.mult,
                                mybir.AluOpType.add,
                            )
                        accs[ci] = nacc
                s01 = accp.tile([128, OUT], fp16, tag="acc0")
                nc.vector.tensor_add(s01[:], accs[0][:], accs[1][:])
                s23 = accp.tile([128, OUT], fp16, tag="acc1")
                nc.vector.tensor_add(s23[:], accs[2][:], accs[3][:])
                acc = accp.tile([128, OUT], f32, tag="acc2")
                nc.vector.tensor_add(acc[:], s01[:], s23[:])
                # softplus(x) = relu(x) + ln(1 + exp(-min(|x|, 87))); abs/exp/
                # ln on ScalarE (one table, pre-warmed), min/relu/adds on DVE.
                t_abs = outp.tile([128, OUT], f32, tag="outp")
                nc.scalar.activation(
                    t_abs[:], acc[:], mybir.ActivationFunctionType.Abs
                )
                t_cl = outp.tile([128, OUT], f32, tag="outp")
                nc.vector.tensor_scalar_min(t_cl[:], t_abs[:], 87.0)
                t_exp = outp.tile([128, OUT], f32, tag="outp")
                nc.scalar.activation(
                    t_exp[:], t_cl[:], mybir.ActivationFunctionType.Exp, scale=-1.0
                )
                t_ln = outp.tile([128, OUT], f32, tag="outp")
                nc.scalar.activation(
                    t_ln[:], t_exp[:], mybir.ActivationFunctionType.Ln, bias=1.0
                )
                t_relu = outp.tile([128, OUT], f32, tag="outp")
                nc.vector.tensor_scalar_max(t_relu[:], acc[:], 0.0)
                t_s = outp.tile([128, OUT], f32, tag="outp")
                nc.vector.tensor_add(t_s[:], t_ln[:], t_relu[:])
                ot = outp.tile([128, OUT], f32, tag="outp")
                nc.vector.tensor_add(ot[:], t_s[:], biassb[:])
                nc.scalar.dma_start(out_d[128 * blk : 128 * (blk + 1), :], ot[:])

    nc.compile()
    return nc


def _prep_inputs(dist_adj, dist_exp, atom_emb, bilinear_w, bilinear_b):
    dist_adj = np.asarray(dist_adj, dtype=np.float32)
    dist_exp = np.asarray(dist_exp, dtype=np.float32)
    atom_emb = np.asarray(atom_emb, dtype=np.float32)
    bilinear_w = np.asarray(bilinear_w, dtype=np.float32)
    bilinear_b = np.asarray(bilinear_b, dtype=np.float32)

    # [core, t, p, aq]: groups of 8 atoms; per partition 16 KiB contiguous.
    # aq = 1024a + 64c + k, n = 16p + c.
    exp_b = (
        dist_exp.astype(_BF)
        .reshape(N_CORES, NG, GA, 128, 1024)
        .transpose(0, 1, 3, 2, 4)
        .reshape(N_CORES, NG, 128, 8192)
    )
    # adjA[core, j, 16m + c] = dist_adj[core*M + m, 16j + c]
    adjA = (
        dist_adj.reshape(N_CORES, M, 128, 16)
        .transpose(0, 2, 1, 3)
        .reshape(N_CORES, 128, 16 * M)
        .astype(_BF, order="C")
    )
    embT = atom_emb.reshape(N_CORES, M, H).transpose(0, 2, 1).astype(_BF, order="C")
    w2 = bilinear_w.transpose(1, 0, 2).reshape(H, K * OUT).astype(_BF, order="C")
    biasb = np.ascontiguousarray(
        np.broadcast_to(bilinear_b.astype(np.float32), (128, OUT))
    )

    in_maps = []
    for i in range(N_CORES):
        in_maps.append(
            {
                "exp": np.ascontiguousarray(exp_b[i]),
                "adjA": np.ascontiguousarray(adjA[i]),
                "embT": np.ascontiguousarray(embT[i]),
                "w": w2,
                "bias": biasb,
            }
        )
    return in_maps


def _run(in_maps, **kwargs):
    _ensure_path()
    from concourse.bass_utils import run_bass_kernel_spmd

    if "nc" not in _CACHE:
        _CACHE["nc"] = _build()
    nc = _CACHE["nc"]
    res = run_bass_kernel_spmd(nc, in_maps, core_ids=list(range(N_CORES)), **kwargs)
    return res


def kernel(dist_adj, dist_exp, atom_emb, bilinear_w, bilinear_b):
    in_maps = _prep_inputs(dist_adj, dist_exp, atom_emb, bilinear_w, bilinear_b)
    res = _run(in_maps)
    out = np.concatenate(
        [np.asarray(res.results[i]["out"]) for i in range(N_CORES)], axis=0
    )
    return out.astype(np.float32)


# revision 14
# speedup vs baseline: 1.2108x; 1.2108x over previous
# Trainium2 Bass kernel for AtomTypeGNN message passing.
#
#   adj_exp[m,k] = sum_n dist_adj[m,n] * dist_exp[m,n,k]          (streams 1 GiB)
#   feat[m,o]    = sum_{f,h} adj_exp[m,f] * w[f,h,o] * emb[m,h]
#   out          = softplus(feat) + b
#
# Output row m depends only on row m of the inputs -> pure data parallel over
# atoms, 8 NeuronCores, 256 atoms each, no collectives.
#
# Device strategy (per core), v2:
#   Step 1: atoms stream in groups of 8; one 2 MiB DMA per group on the sync
#     (SP HWDGE) queue, which carries NOTHING else, so no cross-phase
#     dependency ever stalls the exp stream (v1 lost ~15us to the aexp load
#     parked between blocks in the SP FIFO).  All 8 atoms of a group
#     accumulate into ONE PSUM bank at different free offsets: start=True on
#     the group's first matmul marks the whole 2 KiB zero region pending-zero,
#     so each atom's first chunk writes and later chunks accumulate
#     (has_written semantics).  One ScalarE evac [1,512] per group replaces
#     v1's four [1,64] copies (ScalarE busy ~115us -> ~22us).
#   Step 2: per 128-atom block, ONE scratch store + ONE gather load (both on
#     the gpsimd SWDGE queue) transpose adj_exp to [128 atoms, 64].  The
#     f-contraction feat = sum_f aexp[:,f] * G_f runs as FOUR interleaved
#     DVE scalar_tensor_tensor chains (dep distance 4 hides DVE latency);
#     G_f = emb @ w[f] is computed on the PE during the stream.  Softplus
#     splits between ScalarE (abs/exp/ln in one act table, pre-warmed at
#     kernel start so no table load lands in the tail) and DVE (min/relu/
#     adds).  Output DMAs ride the scalar (ACT HWDGE) queue: gpsimd's
#     end-of-kernel DRAIN cost ~7us in v1.
#
# Inputs are pre-swizzled/cast on the host (bf16 compute, f32 accumulate:
# ~3e-3 relative error, memory roofline halves to ~190us/core).

import numpy as np
import ml_dtypes

N = 2048
K = 64
H = 128
OUT = 128
N_CORES = 8
M = N // N_CORES  # 256 atoms per core
GA = 8            # atoms per group / per PSUM bank
NG = M // GA      # 32 groups per core
NBLK = M // 128   # 2 step-2 blocks per core

_BF = ml_dtypes.bfloat16

_CACHE = {}


def _ensure_path():
    import sys

    for p in ("/opt/trn_rl_repo",):
        if p not in sys.path:
            sys.path.insert(0, p)


def _build():
    _ensure_path()
    import concourse.bass as bass  # noqa: F401
    import concourse.tile as tile
    from concourse import bacc, mybir

    f32 = mybir.dt.float32
    bf16 = mybir.dt.bfloat16
    fp16 = mybir.dt.float16

    nc = bacc.Bacc(
        "TRN2",
        target_bir_lowering=False,
        debug=False,
        num_devices=N_CORES,
    )

    # [t, p, aq]: atom group t = atoms 8t..8t+7, partition p, aq = 1024*a + q,
    # q = 64*c + k, n = 16p + c.  Per partition 16 KiB contiguous in DRAM.
    exp_d = nc.declare_dram_parameter("exp", [NG, 128, 8 * 1024], bf16, isOutput=False)
    # adjA[j, 16m + c] = dist_adj[m, 16j + c]
    adjA_d = nc.declare_dram_parameter("adjA", [128, 16 * M], bf16, isOutput=False)
    # embT[h, m]
    embT_d = nc.declare_dram_parameter("embT", [H, M], bf16, isOutput=False)
    # w2[h, 128f + o] = bilinear_w[f, h, o]
    w_d = nc.declare_dram_parameter("w", [H, K * OUT], bf16, isOutput=False)
    # bias broadcast to all partitions
    bias_d = nc.declare_dram_parameter("bias", [128, OUT], f32, isOutput=False)
    out_d = nc.declare_dram_parameter("out", [M, OUT], f32, isOutput=True)

    # adj_exp bounce buffer, one row per block: [g*512 + a*64 + k]
    scratch_d = nc.dram_tensor("scratch", [NBLK, 16 * 512], fp16)

    with tile.TileContext(nc) as tc:
        with (
            tc.tile_pool(name="const", bufs=1) as constp,
            tc.tile_pool(name="exp", bufs=6) as expp,
            tc.tile_pool(name="ps1", bufs=5, space="PSUM") as ps1p,
            tc.tile_pool(name="stage", bufs=1) as stagep,
            tc.tile_pool(name="aexp", bufs=2) as aexpp,
            tc.tile_pool(name="ps2", bufs=3, space="PSUM") as ps2p,
            tc.tile_pool(name="gsb", bufs=2) as gsbp,
            tc.tile_pool(name="acc", bufs=10) as accp,
            tc.tile_pool(name="outp", bufs=6) as outp,
        ):
            # consts at the HEAD of the sync queue: they must land at full
            # rate before the stream floods HBM (on the scalar queue they
            # trickled at ~70 GB/s against the saturated stream and the PE
            # sat idle 20us waiting for adjA).
            biassb = constp.tile([128, OUT], f32, tag="bias")
            nc.sync.dma_start(biassb[:], bias_d[:, :])
            adjA = constp.tile([128, 16 * M], bf16, tag="adjA")
            nc.sync.dma_start(adjA[:], adjA_d[:, :])
            wsb = constp.tile([128, K * OUT], bf16, tag="wsb")
            nc.sync.dma_start(wsb[:], w_d[:, :])
            embT = constp.tile([128, M], bf16, tag="embT")
            nc.sync.dma_start(embT[:], embT_d[:, :])

            # Warm the natural_log_exp act table (abs/exp/ln/relu/copy share
            # it) before the first evac copy, so no ACT_TABLE_LOAD lands in
            # the tail's critical path.
            warm = constp.tile([1, 2], f32, tag="warm")
            nc.scalar.activation(
                warm[0:1, :], biassb[0:1, 0:2], mybir.ActivationFunctionType.Abs
            )

            for blk in range(NBLK):
                # G_f = (emb @ w[f]) for this block, computed during the
                # stream: one matmul per group covers FOUR f's (a full PSUM
                # bank), evacuated by ScalarE straight to fp16.  DVE carries
                # ONLY the tail chains — giving it G casts (v2) let a chain
                # waiting at the head of the in-order DVE queue starve ps2,
                # which head-of-line-blocked the PE stream.
                gsb = gsbp.tile([128, K * OUT], fp16, tag="gsb")
                # 16 group stages, each [1, 512] = 8 atoms x 64 k
                stage = stagep.tile([1, 16 * 512], fp16, tag="stage")

                for g in range(16):
                    t = blk * 16 + g
                    et = expp.tile([128, 8 * 1024], bf16, tag="exp")
                    nc.sync.dma_start(et[:], exp_d[t])
                    # 8 atoms share one PSUM bank; single start clears the
                    # whole 2 KiB zero region, per-address has_written turns
                    # each atom's first chunk into a write.
                    ps = ps1p.tile([128, 512], f32, tag="ps1")
                    for a in range(GA):
                        m = t * GA + a
                        for c in range(16):
                            nc.tensor.matmul(
                                ps[0:1, 64 * a : 64 * (a + 1)],
                                adjA[:, 16 * m + c : 16 * m + c + 1],
                                et[:, 1024 * a + 64 * c : 1024 * a + 64 * (c + 1)],
                                start=(a == 0 and c == 0),
                                stop=(a == GA - 1 and c == 15),
                            )
                    nc.scalar.copy(stage[0:1, 512 * g : 512 * (g + 1)], ps[0:1, :])
                    g2 = ps2p.tile([128, 4 * OUT], f32, tag="ps2")
                    nc.tensor.matmul(
                        g2[:, :],
                        embT[:, 128 * blk : 128 * (blk + 1)],
                        wsb[:, OUT * 4 * g : OUT * 4 * (g + 1)],
                        start=True,
                        stop=True,
                    )
                    nc.scalar.copy(gsb[:, OUT * 4 * g : OUT * 4 * (g + 1)], g2[:, :])

                # ---- step 2 for this block of 128 atoms ----
                # transpose bounce through DRAM on the gpsimd queue (the sync
                # queue must stay pure exp stream)
                nc.gpsimd.dma_start(scratch_d[blk : blk + 1, :], stage[0:1, :])
                # f32: DVE tensor_scalar ops require a float32 scalar operand;
                # the SWDGE load casts fp16 -> f32 in flight.
                aexp = aexpp.tile([128, K], f32, tag="aexp")
                nc.gpsimd.dma_start(
                    aexp[:],
                    scratch_d[blk : blk + 1, :].rearrange("b (p k) -> (b p) k", k=K),
                )
                # four interleaved DVE scale-accumulate chains over f
                accs = [None] * 4
                for r in range(16):
                    for ci in range(4):
                        f = 4 * r + ci
                        nacc = accp.tile([128, OUT], fp16, tag=f"acc{ci}")
                        if r == 0:
                            nc.vector.tensor_scalar_mul(
                                nacc[:], gsb[:, OUT * f : OUT * (f + 1)],
                                aexp[:, f : f + 1],
                            )
                        else:
                            nc.vector.scalar_tensor_tensor(
                                nacc[:],
                                gsb[:, OUT * f : OUT * (f + 1)],
                                aexp[:, f : f + 1],
                                accs[ci][:],
                                mybir.AluOpType.mult,
                                mybir.AluOpType.add,
                            )
                        accs[ci] = nacc
                s01 = accp.tile([128, OUT], fp16, tag="acc0")
                nc.vector.tensor_add(s01[:], accs[0][:], accs[1][:])
                s23 = accp.tile([128, OUT], fp16, tag="acc1")
                nc.vector.tensor_add(s23[:], accs[2][:], accs[3][:])
                acc = accp.tile([128, OUT], f32, tag="acc2")
                nc.vector.tensor_add(acc[:], s01[:], s23[:])
                # softplus(x) = relu(x) + ln(1 + exp(-min(|x|, 87))); abs/exp/
                # ln on ScalarE (one table, pre-warmed), min/relu/adds on DVE.
                t_abs = outp.tile([128, OUT], f32, tag="outp")
                nc.scalar.activation(
                    t_abs[:], acc[:], mybir.ActivationFunctionType.Abs
                )
                t_cl = outp.tile([128, OUT], f32, tag="outp")
                nc.vector.tensor_scalar_min(t_cl[:], t_abs[:], 87.0)
                t_exp = outp.tile([128, OUT], f32, tag="outp")
                nc.scalar.activation(
                    t_exp[:], t_cl[:], mybir.ActivationFunctionType.Exp, scale=-1.0
                )
                t_ln = outp.tile([128, OUT], f32, tag="outp")
                nc.scalar.activation(
                    t_ln[:], t_exp[:], mybir.ActivationFunctionType.Ln, bias=1.0
                )
                t_relu = outp.tile([128, OUT], f32, tag="outp")
                nc.vector.tensor_scalar_max(t_relu[:], acc[:], 0.0)
                t_s = outp.tile([128, OUT], f32, tag="outp")
                nc.vector.tensor_add(t_s[:], t_ln[:], t_relu[:])
                ot = outp.tile([128, OUT], f32, tag="outp")
                nc.vector.tensor_add(ot[:], t_s[:], biassb[:])
                nc.scalar.dma_start(out_d[128 * blk : 128 * (blk + 1), :], ot[:])

    nc.compile()
    return nc


def _prep_inputs(dist_adj, dist_exp, atom_emb, bilinear_w, bilinear_b):
    dist_adj = np.asarray(dist_adj, dtype=np.float32)
    dist_exp = np.asarray(dist_exp, dtype=np.float32)
    atom_emb = np.asarray(atom_emb, dtype=np.float32)
    bilinear_w = np.asarray(bilinear_w, dtype=np.float32)
    bilinear_b = np.asarray(bilinear_b, dtype=np.float32)

    # [core, t, p, aq]: groups of 8 atoms; per partition 16 KiB contiguous.
    # aq = 1024a + 64c + k, n = 16p + c.
    exp_b = (
        dist_exp.astype(_BF)
        .reshape(N_CORES, NG, GA, 128, 1024)
        .transpose(0, 1, 3, 2, 4)
        .reshape(N_CORES, NG, 128, 8192)
    )
    # adjA[core, j, 16m + c] = dist_adj[core*M + m, 16j + c]
    adjA = (
        dist_adj.reshape(N_CORES, M, 128, 16)
        .transpose(0, 2, 1, 3)
        .reshape(N_CORES, 128, 16 * M)
        .astype(_BF, order="C")
    )
    embT = atom_emb.reshape(N_CORES, M, H).transpose(0, 2, 1).astype(_BF, order="C")
    w2 = bilinear_w.transpose(1, 0, 2).reshape(H, K * OUT).astype(_BF, order="C")
    biasb = np.ascontiguousarray(
        np.broadcast_to(bilinear_b.astype(np.float32), (128, OUT))
    )

    in_maps = []
    for i in range(N_CORES):
        in_maps.append(
            {
                "exp": np.ascontiguousarray(exp_b[i]),
                "adjA": np.ascontiguousarray(adjA[i]),
                "embT": np.ascontiguousarray(embT[i]),
                "w": w2,
                "bias": biasb,
            }
        )
    return in_maps


def _run(in_maps, **kwargs):
    _ensure_path()
    from concourse.bass_utils import run_bass_kernel_spmd

    if "nc" not in _CACHE:
        _CACHE["nc"] = _build()
    nc = _CACHE["nc"]
    res = run_bass_kernel_spmd(nc, in_maps, core_ids=list(range(N_CORES)), **kwargs)
    return res


def kernel(dist_adj, dist_exp, atom_emb, bilinear_w, bilinear_b):
    in_maps = _prep_inputs(dist_adj, dist_exp, atom_emb, bilinear_w, bilinear_b)
    res = _run(in_maps)
    out = np.concatenate(
        [np.asarray(res.results[i]["out"]) for i in range(N_CORES)], axis=0
    )
    return out.astype(np.float32)


# revision 26
# speedup vs baseline: 1.2997x; 1.0734x over previous
# Trainium2 Bass kernel for AtomTypeGNN message passing.
#
#   adj_exp[m,k] = sum_n dist_adj[m,n] * dist_exp[m,n,k]          (streams 1 GiB)
#   feat[m,o]    = sum_{f,h} adj_exp[m,f] * w[f,h,o] * emb[m,h]
#   out          = softplus(feat) + b
#
# Output row m depends only on row m of the inputs -> pure data parallel over
# atoms, 8 NeuronCores, 256 atoms each, no collectives.
#
# Device strategy (per core), v6:
#   Step 1 streams atoms in groups of 8 (one 2 MiB DMA per group on the sync
#   queue, which carries nothing else so no cross-phase dependency can stall
#   the stream).  Per n-chunk c one matmul covers ALL 8 atoms of the group:
#     stationary adjC[:, (t,c)] = [128, 8]  (chunk-c adj columns, 8 atoms)
#     moving     et[:, (a,c,:)] = [128, 8*64] (strided AP)
#     out        ps[8, 512]     (PSUM bank; atom a's true result is the
#                                diagonal block [a, 64a:64a+64], off-diagonal
#                                blocks are discarded cross-atom garbage)
#   16 matmuls per group (vs 256 when each matmul did one atom x one chunk)
#   keeps the whole PE program near 1.1k instructions: the fully unrolled
#   8k-instruction version hit an IRAM 16-KiB block fetch from saturated HBM
#   every 256 instructions, ~2.1us per stall, ~60us per core.
#
#   The diagonal extraction rides the scratch bounce: ScalarE evacuates each
#   group's [8, 512] bank to fp16 stage rows, one DMA stores them to DRAM,
#   and 8 strided loads (one per a) pick the diagonal blocks and land them
#   transposed as aexp[p=16a+g, k].  Step-2 atom order inside a block is
#   therefore m' = 16a + g; the host permutes embT accordingly and
#   inverse-permutes the output rows.
#
#   Step 2: G_f = emb @ w[f] is computed on the PE during the stream (one
#   matmul per group covers four f's = a full PSUM bank) and evacuated by
#   ScalarE straight to fp16.  feat = sum_f aexp[:,f] * G_f runs as FOUR
#   interleaved fp16 DVE scalar_tensor_tensor chains (dep distance 4 hides
#   DVE latency; DVE carries nothing else, so a chain waiting at the head of
#   its in-order queue cannot starve the stream).  Softplus splits between
#   ScalarE (abs/exp/ln in one pre-warmed act table) and DVE (min/relu/adds).
#   Output DMAs ride the scalar (ACT HWDGE) queue; the gpsimd (SWDGE) queue
#   carries only the scratch bounce.
#
# Inputs are pre-swizzled/cast on the host (bf16 stream, f32 accumulate,
# fp16 step-2 tail: ~3.4e-3 relative error; memory roofline ~190us/core).

import numpy as np
import ml_dtypes

N = 2048
K = 64
H = 128
OUT = 128
N_CORES = 8
M = N // N_CORES  # 256 atoms per core
GA = 8            # atoms per group / per PSUM bank
NG = M // GA      # 32 groups per core
NBLK = M // 128   # 2 step-2 blocks per core
SROW = 8704       # scratch row length (>= 8192 + 7*64 so diagonal slices fit)

_BF = ml_dtypes.bfloat16

_CACHE = {}


def _ensure_path():
    import sys

    for p in ("/opt/trn_rl_repo",):
        if p not in sys.path:
            sys.path.insert(0, p)


def _build():
    _ensure_path()
    import concourse.bass as bass  # noqa: F401
    import concourse.tile as tile
    from concourse import bacc, mybir

    f32 = mybir.dt.float32
    bf16 = mybir.dt.bfloat16
    fp16 = mybir.dt.float16

    nc = bacc.Bacc(
        "TRN2",
        target_bir_lowering=False,
        debug=False,
        num_devices=N_CORES,
    )

    # [t, p, aq]: atom group t = atoms 8t..8t+7, partition p, aq = 1024*a + q,
    # q = 64*c + k, n = 16p + c.  Per partition 16 KiB contiguous in DRAM.
    exp_d = nc.declare_dram_parameter("exp", [NG, 128, 8 * 1024], bf16, isOutput=False)
    # adjC[j, 128t + 8c + a] = dist_adj[8t + a, 16j + c]
    adjC_d = nc.declare_dram_parameter("adjC", [128, 16 * M], bf16, isOutput=False)
    # embT[h, m'] with within-block order m' = 16a + g
    embT_d = nc.declare_dram_parameter("embT", [H, M], bf16, isOutput=False)
    # w2[h, 128f + o] = bilinear_w[f, h, o]
    w_d = nc.declare_dram_parameter("w", [H, K * OUT], bf16, isOutput=False)
    # bias broadcast to all partitions
    bias_d = nc.declare_dram_parameter("bias", [128, OUT], f32, isOutput=False)
    # rows ordered m' = 16a + g within each block; host inverse-permutes
    out_d = nc.declare_dram_parameter("out", [M, OUT], f32, isOutput=True)

    # adj_exp bounce, [blk, a, 512g + 64a + k] (diagonal picked at load time)
    scratch_d = nc.dram_tensor("scratch", [NBLK, GA, SROW], fp16)

    with tile.TileContext(nc) as tc:
        with (
            tc.tile_pool(name="const", bufs=1) as constp,
            tc.tile_pool(name="exp", bufs=6) as expp,
            tc.tile_pool(name="ps1", bufs=4, space="PSUM") as ps1p,
            tc.tile_pool(name="stage", bufs=1) as stagep,
            tc.tile_pool(name="aexp", bufs=2) as aexpp,
            tc.tile_pool(name="ps2", bufs=3, space="PSUM") as ps2p,
            tc.tile_pool(name="gsb", bufs=2) as gsbp,
            tc.tile_pool(name="acc", bufs=10) as accp,
            tc.tile_pool(name="outp", bufs=6) as outp,
        ):
            # consts at the HEAD of the sync queue: they must land at full
            # rate before the stream floods HBM (on the scalar queue they
            # trickled at ~70 GB/s against the saturated stream and the PE
            # sat idle 20us waiting for weights).
            biassb = constp.tile([128, OUT], f32, tag="bias")
            nc.sync.dma_start(biassb[:], bias_d[:, :])
            adjC = constp.tile([128, 16 * M], bf16, tag="adjC")
            nc.sync.dma_start(adjC[:], adjC_d[:, :])
            wsb = constp.tile([128, K * OUT], bf16, tag="wsb")
            nc.sync.dma_start(wsb[:], w_d[:, :])
            embT = constp.tile([128, M], bf16, tag="embT")
            nc.sync.dma_start(embT[:], embT_d[:, :])

            # Warm the natural_log_exp act table (abs/exp/ln/relu/copy share
            # it) before the first evac copy, so no ACT_TABLE_LOAD lands in
            # the tail's critical path.
            warm = constp.tile([1, 2], f32, tag="warm")
            nc.scalar.activation(
                warm[0:1, :], biassb[0:1, 0:2], mybir.ActivationFunctionType.Abs
            )

            for blk in range(NBLK):
                gsb = gsbp.tile([128, K * OUT], fp16, tag="gsb")
                # 16 group stages, each [8, 512]
                stage = stagep.tile([GA, 16 * 512], fp16, tag="stage")

                for g in range(16):
                    t = blk * 16 + g
                    et = expp.tile([128, 8 * 1024], bf16, tag="exp")
                    nc.sync.dma_start(et[:], exp_d[t])
                    et_ak = et[:].rearrange("p (a x) -> p a x", a=GA)
                    ps = ps1p.tile([GA, 512], f32, tag="ps1")
                    for c in range(16):
                        nc.tensor.matmul(
                            ps[:, :],
                            adjC[:, 128 * t + 8 * c : 128 * t + 8 * (c + 1)],
                            et_ak[:, :, 64 * c : 64 * (c + 1)],
                            start=(c == 0),
                            stop=(c == 15),
                        )
                    nc.scalar.copy(stage[:, 512 * g : 512 * (g + 1)], ps[:, :])
                    # one G matmul per group covers four f's (a full bank)
                    g2 = ps2p.tile([128, 4 * OUT], f32, tag="ps2")
                    nc.tensor.matmul(
                        g2[:, :],
                        embT[:, 128 * blk : 128 * (blk + 1)],
                        wsb[:, OUT * 4 * g : OUT * 4 * (g + 1)],
                        start=True,
                        stop=True,
                    )
                    nc.scalar.copy(gsb[:, OUT * 4 * g : OUT * 4 * (g + 1)], g2[:, :])

                # ---- step 2 for this block of 128 atoms ----
                # bounce through DRAM on the gpsimd queue; the 8 loads pick
                # atom a's diagonal blocks [a, 512g + 64a + k] and land them
                # at partitions p = 16a + g.
                nc.gpsimd.dma_start(scratch_d[blk, :, 0 : 16 * 512], stage[:, :])
                aexp = aexpp.tile([128, K], f32, tag="aexp")
                for a in range(GA):
                    src = scratch_d[blk, a : a + 1, 64 * a : 64 * a + 8192]
                    src = src.rearrange("one (g x) -> (one g) x", x=512)
                    nc.gpsimd.dma_start(aexp[16 * a : 16 * (a + 1), :], src[:, 0:K])
                # four interleaved fp16 DVE scale-accumulate chains over f
                accs = [None] * 4
                for r in range(16):
                    for ci in range(4):
                        f = 4 * r + ci
                        nacc = accp.tile([128, OUT], fp16, tag=f"acc{ci}")
                        if r == 0:
                            nc.vector.tensor_scalar_mul(
                                nacc[:], gsb[:, OUT * f : OUT * (f + 1)],
                                aexp[:, f : f + 1],
                            )
                        else:
                            nc.vector.scalar_tensor_tensor(
                                nacc[:],
                                gsb[:, OUT * f : OUT * (f + 1)],
                                aexp[:, f : f + 1],
                                accs[ci][:],
                                mybir.AluOpType.mult,
                                mybir.AluOpType.add,
                            )
                        accs[ci] = nacc
                s01 = accp.tile([128, OUT], fp16, tag="acc0")
                nc.vector.tensor_add(s01[:], accs[0][:], accs[1][:])
                s23 = accp.tile([128, OUT], fp16, tag="acc1")
                nc.vector.tensor_add(s23[:], accs[2][:], accs[3][:])
                acc = accp.tile([128, OUT], f32, tag="acc2")
                nc.vector.tensor_add(acc[:], s01[:], s23[:])
                # softplus(x) = relu(x) + ln(1 + exp(-min(|x|, 87))); abs/exp/
                # ln on ScalarE (one table, pre-warmed), min/relu/adds on DVE.
                t_abs = outp.tile([128, OUT], f32, tag="outp")
                nc.scalar.activation(
                    t_abs[:], acc[:], mybir.ActivationFunctionType.Abs
                )
                t_cl = outp.tile([128, OUT], f32, tag="outp")
                nc.vector.tensor_scalar_min(t_cl[:], t_abs[:], 87.0)
                t_exp = outp.tile([128, OUT], f32, tag="outp")
                nc.scalar.activation(
                    t_exp[:], t_cl[:], mybir.ActivationFunctionType.Exp, scale=-1.0
                )
                t_ln = outp.tile([128, OUT], f32, tag="outp")
                nc.scalar.activation(
                    t_ln[:], t_exp[:], mybir.ActivationFunctionType.Ln, bias=1.0
                )
                t_relu = outp.tile([128, OUT], f32, tag="outp")
                nc.vector.tensor_scalar_max(t_relu[:], acc[:], 0.0)
                t_s = outp.tile([128, OUT], f32, tag="outp")
                nc.vector.tensor_add(t_s[:], t_ln[:], t_relu[:])
                ot = outp.tile([128, OUT], f32, tag="outp")
                nc.vector.tensor_add(ot[:], t_s[:], biassb[:])
                nc.scalar.dma_start(out_d[128 * blk : 128 * (blk + 1), :], ot[:])

    nc.compile()
    return nc


# within-block atom permutation: step-2 partition p = 16a + g holds the
# block's atom 8g + a
_PERM = np.array([8 * (p % 16) + p // 16 for p in range(128)])


def _prep_inputs(dist_adj, dist_exp, atom_emb, bilinear_w, bilinear_b):
    dist_adj = np.asarray(dist_adj, dtype=np.float32)
    dist_exp = np.asarray(dist_exp, dtype=np.float32)
    atom_emb = np.asarray(atom_emb, dtype=np.float32)
    bilinear_w = np.asarray(bilinear_w, dtype=np.float32)
    bilinear_b = np.asarray(bilinear_b, dtype=np.float32)

    # [core, t, p, aq]: groups of 8 atoms; per partition 16 KiB contiguous.
    # aq = 1024a + 64c + k, n = 16p + c.
    exp_b = (
        dist_exp.astype(_BF)
        .reshape(N_CORES, NG, GA, 128, 1024)
        .transpose(0, 1, 3, 2, 4)
        .reshape(N_CORES, NG, 128, 8192)
    )
    # adjC[core, j, 128t + 8c + a] = dist_adj[core*M + 8t + a, 16j + c]
    adjC = (
        dist_adj.reshape(N_CORES, NG, GA, 128, 16)
        .transpose(0, 3, 1, 4, 2)
        .reshape(N_CORES, 128, 16 * M)
        .astype(_BF, order="C")
    )
    # embT[core, h, m'] with block rows permuted to m' = 16a + g
    emb_p = (
        atom_emb.reshape(N_CORES, NBLK, 128, H)[:, :, _PERM, :]
        .reshape(N_CORES, M, H)
    )
    embT = emb_p.transpose(0, 2, 1).astype(_BF, order="C")
    w2 = bilinear_w.transpose(1, 0, 2).reshape(H, K * OUT).astype(_BF, order="C")
    biasb = np.ascontiguousarray(
        np.broadcast_to(bilinear_b.astype(np.float32), (128, OUT))
    )

    in_maps = []
    for i in range(N_CORES):
        in_maps.append(
            {
                "exp": np.ascontiguousarray(exp_b[i]),
                "adjC": np.ascontiguousarray(adjC[i]),
                "embT": np.ascontiguousarray(embT[i]),
                "w": w2,
                "bias": biasb,
            }
        )
    return in_maps


def _run(in_maps, **kwargs):
    _ensure_path()
    from concourse.bass_utils import run_bass_kernel_spmd

    if "nc" not in _CACHE:
        _CACHE["nc"] = _build()
    nc = _CACHE["nc"]
    res = run_bass_kernel_spmd(nc, in_maps, core_ids=list(range(N_CORES)), **kwargs)
    return res


def kernel(dist_adj, dist_exp, atom_emb, bilinear_w, bilinear_b):
    in_maps = _prep_inputs(dist_adj, dist_exp, atom_emb, bilinear_w, bilinear_b)
    res = _run(in_maps)
    out = np.concatenate(
        [np.asarray(res.results[i]["out"]) for i in range(N_CORES)], axis=0
    )
    # undo the within-block atom permutation (row m' = 16a+g is atom 8g+a)
    inv = np.argsort(_PERM)
    out = out.reshape(2 * N_CORES, 128, OUT)[:, inv, :].reshape(N, OUT)
    return out.astype(np.float32)


# revision 28
# speedup vs baseline: 1.3345x; 1.0268x over previous
# Trainium2 Bass kernel for AtomTypeGNN message passing.
#
#   adj_exp[m,k] = sum_n dist_adj[m,n] * dist_exp[m,n,k]          (streams 1 GiB)
#   feat[m,o]    = sum_{f,h} adj_exp[m,f] * w[f,h,o] * emb[m,h]
#   out          = softplus(feat) + b
#
# Output row m depends only on row m of the inputs -> pure data parallel over
# atoms, 8 NeuronCores, 256 atoms each, no collectives.
#
# Device strategy (per core), v6:
#   Step 1 streams atoms in groups of 8 (one 2 MiB DMA per group on the sync
#   queue, which carries nothing else so no cross-phase dependency can stall
#   the stream).  Per n-chunk c one matmul covers ALL 8 atoms of the group:
#     stationary adjC[:, (t,c)] = [128, 8]  (chunk-c adj columns, 8 atoms)
#     moving     et[:, (a,c,:)] = [128, 8*64] (strided AP)
#     out        ps[8, 512]     (PSUM bank; atom a's true result is the
#                                diagonal block [a, 64a:64a+64], off-diagonal
#                                blocks are discarded cross-atom garbage)
#   16 matmuls per group (vs 256 when each matmul did one atom x one chunk)
#   keeps the whole PE program near 1.1k instructions: the fully unrolled
#   8k-instruction version hit an IRAM 16-KiB block fetch from saturated HBM
#   every 256 instructions, ~2.1us per stall, ~60us per core.
#
#   The diagonal extraction rides the scratch bounce: ScalarE evacuates each
#   group's [8, 512] bank to fp16 stage rows, one DMA stores them to DRAM,
#   and 8 strided loads (one per a) pick the diagonal blocks and land them
#   transposed as aexp[p=16a+g, k].  Step-2 atom order inside a block is
#   therefore m' = 16a + g; the host permutes embT accordingly and
#   inverse-permutes the output rows.
#
#   Step 2: G_f = emb @ w[f] is computed on the PE during the stream (one
#   matmul per group covers four f's = a full PSUM bank) and evacuated by
#   ScalarE straight to fp16.  feat = sum_f aexp[:,f] * G_f runs as FOUR
#   interleaved fp16 DVE scalar_tensor_tensor chains (dep distance 4 hides
#   DVE latency; DVE carries nothing else, so a chain waiting at the head of
#   its in-order queue cannot starve the stream).  Softplus splits between
#   ScalarE (abs/exp/ln in one pre-warmed act table) and DVE (min/relu/adds).
#   Output DMAs ride the scalar (ACT HWDGE) queue; the gpsimd (SWDGE) queue
#   carries only the scratch bounce.
#
# Inputs are pre-swizzled/cast on the host (bf16 stream, f32 accumulate,
# fp16 step-2 tail: ~3.4e-3 relative error; memory roofline ~190us/core).

import numpy as np
import ml_dtypes

N = 2048
K = 64
H = 128
OUT = 128
N_CORES = 8
M = N // N_CORES  # 256 atoms per core
GA = 8            # atoms per group / per PSUM bank
NG = M // GA      # 32 groups per core
NBLK = M // 128   # 2 step-2 blocks per core
SROW = 8704       # scratch row length (>= 8192 + 7*64 so diagonal slices fit)

_BF = ml_dtypes.bfloat16

_CACHE = {}


def _ensure_path():
    import sys

    for p in ("/opt/trn_rl_repo",):
        if p not in sys.path:
            sys.path.insert(0, p)


def _build():
    _ensure_path()
    import concourse.bass as bass  # noqa: F401
    import concourse.tile as tile
    from concourse import bacc, mybir

    f32 = mybir.dt.float32
    bf16 = mybir.dt.bfloat16
    fp16 = mybir.dt.float16

    nc = bacc.Bacc(
        "TRN2",
        target_bir_lowering=False,
        debug=False,
        num_devices=N_CORES,
    )

    # [t, p, aq]: atom group t = atoms 8t..8t+7, partition p, aq = 1024*a + q,
    # q = 64*c + k, n = 16p + c.  Per partition 16 KiB contiguous in DRAM.
    exp_d = nc.declare_dram_parameter("exp", [NG, 128, 8 * 1024], bf16, isOutput=False)
    # adjC[j, 128t + 8c + a] = dist_adj[8t + a, 16j + c]
    adjC_d = nc.declare_dram_parameter("adjC", [128, 16 * M], bf16, isOutput=False)
    # embT[h, m'] with within-block order m' = 16a + g
    embT_d = nc.declare_dram_parameter("embT", [H, M], bf16, isOutput=False)
    # w2[h, 128f + o] = bilinear_w[f, h, o]
    w_d = nc.declare_dram_parameter("w", [H, K * OUT], bf16, isOutput=False)
    # bias broadcast to all partitions
    bias_d = nc.declare_dram_parameter("bias", [128, OUT], f32, isOutput=False)
    # rows ordered m' = 16a + g within each block; host inverse-permutes
    out_d = nc.declare_dram_parameter("out", [M, OUT], f32, isOutput=True)

    # adj_exp bounce, [blk, a, 512g + 64a + k] (diagonal picked at load time)
    scratch_d = nc.dram_tensor("scratch", [NBLK, GA, SROW], fp16)

    with tile.TileContext(nc) as tc:
        with (
            tc.tile_pool(name="const", bufs=1) as constp,
            tc.tile_pool(name="exp", bufs=6) as expp,
            tc.tile_pool(name="ps1", bufs=4, space="PSUM") as ps1p,
            tc.tile_pool(name="stage", bufs=1) as stagep,
            tc.tile_pool(name="aexp", bufs=2) as aexpp,
            tc.tile_pool(name="ps2", bufs=3, space="PSUM") as ps2p,
            tc.tile_pool(name="gsb", bufs=2) as gsbp,
            tc.tile_pool(name="acc", bufs=10) as accp,
            tc.tile_pool(name="outp", bufs=6) as outp,
        ):
            # consts at the HEAD of the sync queue: they must land at full
            # rate before the stream floods HBM (on the scalar queue they
            # trickled at ~70 GB/s against the saturated stream and the PE
            # sat idle 20us waiting for weights).
            biassb = constp.tile([128, OUT], f32, tag="bias")
            nc.sync.dma_start(biassb[:], bias_d[:, :])
            adjC = constp.tile([128, 16 * M], bf16, tag="adjC")
            nc.sync.dma_start(adjC[:], adjC_d[:, :])
            wsb = constp.tile([128, K * OUT], bf16, tag="wsb")
            nc.sync.dma_start(wsb[:], w_d[:, :])
            embT = constp.tile([128, M], bf16, tag="embT")
            nc.sync.dma_start(embT[:], embT_d[:, :])

            # Warm the natural_log_exp act table (abs/exp/ln/relu/copy share
            # it) before the first evac copy, so no ACT_TABLE_LOAD lands in
            # the tail's critical path.
            warm = constp.tile([1, 2], f32, tag="warm")
            nc.scalar.activation(
                warm[0:1, :], biassb[0:1, 0:2], mybir.ActivationFunctionType.Abs
            )
            nc.scalar.activation(
                warm[0:1, :], biassb[0:1, 0:2], mybir.ActivationFunctionType.Exp
            )
            nc.scalar.activation(
                warm[0:1, :], biassb[0:1, 0:2],
                mybir.ActivationFunctionType.Ln, bias=1.0,
            )

            for blk in range(NBLK):
                gsb = gsbp.tile([128, K * OUT], fp16, tag="gsb")
                # 16 group stages, each [8, 512]
                stage = stagep.tile([GA, 16 * 512], fp16, tag="stage")

                for g in range(16):
                    t = blk * 16 + g
                    et = expp.tile([128, 8 * 1024], bf16, tag="exp")
                    nc.sync.dma_start(et[:], exp_d[t])
                    et_ak = et[:].rearrange("p (a x) -> p a x", a=GA)
                    ps = ps1p.tile([GA, 512], f32, tag="ps1")
                    for c in range(16):
                        nc.tensor.matmul(
                            ps[:, :],
                            adjC[:, 128 * t + 8 * c : 128 * t + 8 * (c + 1)],
                            et_ak[:, :, 64 * c : 64 * (c + 1)],
                            start=(c == 0),
                            stop=(c == 15),
                        )
                    nc.scalar.copy(stage[:, 512 * g : 512 * (g + 1)], ps[:, :])
                    # one G matmul per group covers four f's (a full bank)
                    g2 = ps2p.tile([128, 4 * OUT], f32, tag="ps2")
                    nc.tensor.matmul(
                        g2[:, :],
                        embT[:, 128 * blk : 128 * (blk + 1)],
                        wsb[:, OUT * 4 * g : OUT * 4 * (g + 1)],
                        start=True,
                        stop=True,
                    )
                    nc.scalar.copy(gsb[:, OUT * 4 * g : OUT * 4 * (g + 1)], g2[:, :])

                # ---- step 2 for this block of 128 atoms ----
                # bounce through DRAM on the gpsimd queue; the 8 loads pick
                # atom a's diagonal blocks [a, 512g + 64a + k] and land them
                # at partitions p = 16a + g.
                nc.gpsimd.dma_start(scratch_d[blk, :, 0 : 16 * 512], stage[:, :])
                aexp = aexpp.tile([128, K], f32, tag="aexp")
                for a in range(GA):
                    src = scratch_d[blk, a : a + 1, 64 * a : 64 * a + 8192]
                    src = src.rearrange("one (g x) -> (one g) x", x=512)
                    nc.gpsimd.dma_start(aexp[16 * a : 16 * (a + 1), :], src[:, 0:K])
                # eight interleaved fp16 DVE scale-accumulate chains over f
                NCH = 8
                accs = [None] * NCH
                for r in range(K // NCH):
                    for ci in range(NCH):
                        f = NCH * r + ci
                        nacc = accp.tile([128, OUT], fp16, tag=f"acc{ci}")
                        if r == 0:
                            nc.vector.tensor_scalar_mul(
                                nacc[:], gsb[:, OUT * f : OUT * (f + 1)],
                                aexp[:, f : f + 1],
                            )
                        else:
                            nc.vector.scalar_tensor_tensor(
                                nacc[:],
                                gsb[:, OUT * f : OUT * (f + 1)],
                                aexp[:, f : f + 1],
                                accs[ci][:],
                                mybir.AluOpType.mult,
                                mybir.AluOpType.add,
                            )
                        accs[ci] = nacc
                # pairwise merge tree in fp16, final level to f32
                lvl = accs
                while len(lvl) > 2:
                    nxt = []
                    for i in range(0, len(lvl), 2):
                        s = accp.tile([128, OUT], fp16, tag=f"m{i}")
                        nc.vector.tensor_add(s[:], lvl[i][:], lvl[i + 1][:])
                        nxt.append(s)
                    lvl = nxt
                acc = accp.tile([128, OUT], f32, tag="accf")
                nc.vector.tensor_add(acc[:], lvl[0][:], lvl[1][:])
                # softplus(x) = relu(x) + ln(1 + exp(-min(|x|, 87))); abs/
                # exp/ln on ScalarE, min/relu/adds on DVE
                t_abs = outp.tile([128, OUT], f32, tag="outp")
                nc.scalar.activation(
                    t_abs[:], acc[:], mybir.ActivationFunctionType.Abs
                )
                t_cl = outp.tile([128, OUT], f32, tag="outp")
                nc.vector.tensor_scalar_min(t_cl[:], t_abs[:], 87.0)
                t_exp = outp.tile([128, OUT], f32, tag="outp")
                nc.scalar.activation(
                    t_exp[:], t_cl[:], mybir.ActivationFunctionType.Exp, scale=-1.0
                )
                t_ln = outp.tile([128, OUT], f32, tag="outp")
                nc.scalar.activation(
                    t_ln[:], t_exp[:], mybir.ActivationFunctionType.Ln, bias=1.0
                )
                t_relu = outp.tile([128, OUT], f32, tag="outp")
                nc.vector.tensor_scalar_max(t_relu[:], acc[:], 0.0)
                t_s = outp.tile([128, OUT], f32, tag="outp")
                nc.vector.tensor_add(t_s[:], t_ln[:], t_relu[:])
                ot = outp.tile([128, OUT], f32, tag="outp")
                nc.vector.tensor_add(ot[:], t_s[:], biassb[:])
                nc.scalar.dma_start(out_d[128 * blk : 128 * (blk + 1), :], ot[:])

    nc.compile()
    return nc


# within-block atom permutation: step-2 partition p = 16a + g holds the
# block's atom 8g + a
_PERM = np.array([8 * (p % 16) + p // 16 for p in range(128)])


def _prep_inputs(dist_adj, dist_exp, atom_emb, bilinear_w, bilinear_b):
    dist_adj = np.asarray(dist_adj, dtype=np.float32)
    dist_exp = np.asarray(dist_exp, dtype=np.float32)
    atom_emb = np.asarray(atom_emb, dtype=np.float32)
    bilinear_w = np.asarray(bilinear_w, dtype=np.float32)
    bilinear_b = np.asarray(bilinear_b, dtype=np.float32)

    # [core, t, p, aq]: groups of 8 atoms; per partition 16 KiB contiguous.
    # aq = 1024a + 64c + k, n = 16p + c.
    exp_b = (
        dist_exp.astype(_BF)
        .reshape(N_CORES, NG, GA, 128, 1024)
        .transpose(0, 1, 3, 2, 4)
        .reshape(N_CORES, NG, 128, 8192)
    )
    # adjC[core, j, 128t + 8c + a] = dist_adj[core*M + 8t + a, 16j + c]
    adjC = (
        dist_adj.reshape(N_CORES, NG, GA, 128, 16)
        .transpose(0, 3, 1, 4, 2)
        .reshape(N_CORES, 128, 16 * M)
        .astype(_BF, order="C")
    )
    # embT[core, h, m'] with block rows permuted to m' = 16a + g
    emb_p = (
        atom_emb.reshape(N_CORES, NBLK, 128, H)[:, :, _PERM, :]
        .reshape(N_CORES, M, H)
    )
    embT = emb_p.transpose(0, 2, 1).astype(_BF, order="C")
    w2 = bilinear_w.transpose(1, 0, 2).reshape(H, K * OUT).astype(_BF, order="C")
    biasb = np.ascontiguousarray(
        np.broadcast_to(bilinear_b.astype(np.float32), (128, OUT))
    )

    in_maps = []
    for i in range(N_CORES):
        in_maps.append(
            {
                "exp": np.ascontiguousarray(exp_b[i]),
                "adjC": np.ascontiguousarray(adjC[i]),
                "embT": np.ascontiguousarray(embT[i]),
                "w": w2,
                "bias": biasb,
            }
        )
    return in_maps


def _run(in_maps, **kwargs):
    _ensure_path()
    from concourse.bass_utils import run_bass_kernel_spmd

    if "nc" not in _CACHE:
        _CACHE["nc"] = _build()
    nc = _CACHE["nc"]
    res = run_bass_kernel_spmd(nc, in_maps, core_ids=list(range(N_CORES)), **kwargs)
    return res


def kernel(dist_adj, dist_exp, atom_emb, bilinear_w, bilinear_b):
    in_maps = _prep_inputs(dist_adj, dist_exp, atom_emb, bilinear_w, bilinear_b)
    res = _run(in_maps)
    out = np.concatenate(
        [np.asarray(res.results[i]["out"]) for i in range(N_CORES)], axis=0
    )
    # undo the within-block atom permutation (row m' = 16a+g is atom 8g+a)
    inv = np.argsort(_PERM)
    out = out.reshape(2 * N_CORES, 128, OUT)[:, inv, :].reshape(N, OUT)
    return out.astype(np.float32)
